# revision 12
# baseline (speedup 1.0000x reference)
"""Trainium2 Bass kernel for nn_BatchedGatedConvExperts.

Data-parallel over N across 8 cores (core k handles batch n=k).

v2: depthwise 7x7 conv runs on the PE array as per-channel band-matrix
matmuls: contraction over (row-tap i, q_in) = 7*16=112 (+1 ones-row for
bias), rhs = host-prepared p-shifted copies of x^T, lhsT = host-prepared
band matrices w[e,c,i,qi-qo+3]. Output lands as [q_out, (l,p)] per channel,
packed 3 channels per [96,256] psum slab (PSUM write base must be 0/32/64),
staged via ACT to SBUF and DMA-flattened to a DRAM scratch in channel-major
[(e c), (q,l,p)] layout. Phase 2 (GroupNorm -> cond affine -> pw_in+SiLU
gate -> pw_out -> residual) runs in (q,l,p) spatial order; the final
residual add writes (l,p,q) order via a permuted output AP.

Flat-chunk quirk of the reference (torch .chunk on flat E*2C axis):
  silu input for output-expert e = pw_in block (e//2), rows (e%2)*96..+96,
  computed from y2 of expert e//2; gate half from block 4+e//2 / y2[4+e//2].
Experts processed as pairs b in 0..3: y2[b], y2[4+b] -> outputs 2b, 2b+1.
"""
import sys

sys.path.insert(0, "/opt/trn_rl_repo")

import numpy as np

E, C, KS, CONDC = 8, 96, 7, 32
N, L, P = 8, 16, 16
PAD = KS // 2
S = L * P * P  # 4096
EC = E * C  # 768
EPS = 1e-5
NCHUNK = 512
NCH = S // NCHUNK  # 8
KDW = KS * P + 1  # 113: (i, q_in) + ones row
LP = L * P  # 256

_BUILT = None


def _build():
    import concourse.bacc as bacc
    import concourse.mybir as mybir
    from concourse.masks import make_identity
    from concourse.tile import TileContext

    dt = mybir.dt
    f32 = dt.float32
    Alu = mybir.AluOpType
    Act = mybir.ActivationFunctionType

    nc = bacc.Bacc(None, target_bir_lowering=False)

    xq_d = nc.declare_dram_parameter("x_qlp", [C, S], f32, isOutput=False)
    condq_d = nc.declare_dram_parameter("cond_qlp", [CONDC, S], f32, isOutput=False)
    rhs_d = nc.declare_dram_parameter("dw_rhs", [KDW, C * LP], f32, isOutput=False)
    band_d = nc.declare_dram_parameter("dw_band", [KDW, EC * P], f32, isOutput=False)
    gnw_d = nc.declare_dram_parameter("gn_w", [EC], f32, isOutput=False)
    gnb_d = nc.declare_dram_parameter("gn_b", [EC], f32, isOutput=False)
    piw_d = nc.declare_dram_parameter("pw_in_w", [2 * EC, C], f32, isOutput=False)
    pib_d = nc.declare_dram_parameter("pw_in_b", [2 * EC], f32, isOutput=False)
    pow_d = nc.declare_dram_parameter("pw_out_w", [EC, C], f32, isOutput=False)
    pob_d = nc.declare_dram_parameter("pw_out_b", [EC], f32, isOutput=False)
    cw_d = nc.declare_dram_parameter("cond_w", [2 * EC, CONDC], f32, isOutput=False)
    cb_d = nc.declare_dram_parameter("cond_b", [2 * EC], f32, isOutput=False)
    out_d = nc.declare_dram_parameter("out", [EC, S], f32, isOutput=True)

    with TileContext(nc) as tc:
        # per-expert DRAM scratch for dw output, [(c), (q,l,p)]
        from contextlib import ExitStack
        _stk = ExitStack()
        dram = _stk.enter_context(tc.tile_pool(name="dram", bufs=1, space="DRAM"))
        y_scr = [dram.tile([C, S], f32, name=f"y_scr{e}", tag=f"yscr{e}")
                 for e in range(E)]

        # ---------------- phase 1: depthwise conv on PE ----------------
        with tc.tile_pool(name="p1", bufs=1) as p1, \
             tc.tile_pool(name="p1w", bufs=4) as p1w, \
             tc.tile_pool(name="ps1", bufs=4, space="PSUM") as ps1:
            rhs = p1.tile([KDW, C * LP], f32)
            nc.sync.dma_start(out=rhs, in_=rhs_d[:])
            band = p1.tile([KDW, EC * P], f32)
            nc.sync.dma_start(out=band, in_=band_d[:])

            for t3 in range(EC // 3):  # 256 slabs of 3 channels
                pslab = ps1.tile([96, LP], f32, tag="dwps", name="pslab")
                for k in range(3):
                    fc = 3 * t3 + k  # flat (e,c) channel
                    c = fc % C
                    nc.tensor.matmul(
                        pslab[32 * k:32 * k + P, :],
                        band[:, fc * P:(fc + 1) * P],
                        rhs[:, c * LP:(c + 1) * LP],
                        start=True, stop=True)
                stg = p1w.tile([96, LP], f32, tag="stg", name="stg")
                nc.scalar.copy(stg, pslab)
                for k in range(3):
                    fc = 3 * t3 + k
                    e, c = fc // C, fc % C
                    nc.sync.dma_start(
                        out=y_scr[e][c:c + 1, :].rearrange("o (q lp) -> o q lp", q=P),
                        in_=stg[32 * k:32 * k + P, :])

        # ---------------- phase 2 ----------------
        with tc.tile_pool(name="wt", bufs=1) as wt, \
             tc.tile_pool(name="big", bufs=1) as big, \
             tc.tile_pool(name="y2p", bufs=3) as y2p, \
             tc.tile_pool(name="work", bufs=2) as work, \
             tc.tile_pool(name="small", bufs=4) as small, \
             tc.tile_pool(name="ps", bufs=4, space="PSUM") as ps, \
             tc.tile_pool(name="ps_s", bufs=1, space="PSUM") as ps_s:

            ident = wt.tile([128, 128], f32)
            make_identity(nc, ident)

            gn_w = wt.tile([C, E], f32)
            nc.sync.dma_start(out=gn_w, in_=gnw_d[:].rearrange("(e c) -> c e", e=E))
            gn_b = wt.tile([C, E], f32)
            nc.sync.dma_start(out=gn_b, in_=gnb_d[:].rearrange("(e c) -> c e", e=E))
            cb_g = wt.tile([C, E], f32)
            nc.sync.dma_start(out=cb_g, in_=cb_d[:EC].rearrange("(e c) -> c e", e=E))
            cb_b = wt.tile([C, E], f32)
            nc.sync.dma_start(out=cb_b, in_=cb_d[EC:].rearrange("(e c) -> c e", e=E))

            lhsT_in = wt.tile([C + 1, 2 * EC], f32)
            lhsT_out = wt.tile([C + 1, EC], f32)
            lhsT_c = wt.tile([CONDC, 2 * EC], f32)
            for t in range(2 * EC // 128):
                w_raw = work.tile([128, C], f32, tag="wraw", name="wraw")
                nc.sync.dma_start(out=w_raw, in_=piw_d[t * 128:(t + 1) * 128, :])
                pt = ps.tile([C, 128], f32, tag="wtr", bufs=2, name="pt")
                nc.tensor.transpose(pt, w_raw, ident)
                nc.vector.tensor_copy(lhsT_in[0:C, t * 128:(t + 1) * 128], pt)
            for t in range(EC // 128):
                w_raw = work.tile([128, C], f32, tag="wraw", name="wraw")
                nc.sync.dma_start(out=w_raw, in_=pow_d[t * 128:(t + 1) * 128, :])
                pt = ps.tile([C, 128], f32, tag="wtr", bufs=2, name="pt")
                nc.tensor.transpose(pt, w_raw, ident)
                nc.vector.tensor_copy(lhsT_out[0:C, t * 128:(t + 1) * 128], pt)
            for t in range(2 * EC // 128):
                w_raw = work.tile([128, CONDC], f32, tag="wraw", name="wraw")
                nc.sync.dma_start(out=w_raw, in_=cw_d[t * 128:(t + 1) * 128, :])
                pt = ps.tile([CONDC, 128], f32, tag="wtr", bufs=2, name="pt")
                nc.tensor.transpose(pt, w_raw, ident)
                nc.vector.tensor_copy(lhsT_c[:, t * 128:(t + 1) * 128], pt)
            nc.sync.dma_start(out=lhsT_in[C:C + 1, :], in_=pib_d[:])
            nc.sync.dma_start(out=lhsT_out[C:C + 1, :], in_=pob_d[:])

            cond_sb = big.tile([CONDC, S], f32)
            nc.sync.dma_start(out=cond_sb, in_=condq_d[:])
            x_sb = big.tile([C, S], f32)
            nc.sync.dma_start(out=x_sb, in_=xq_d[:])

            ones96 = wt.tile([C, 1], f32)
            nc.vector.memset(ones96, 1.0)
            eps11 = wt.tile([1, 1], f32)
            nc.vector.memset(eps11, EPS)

            def build_y2(e):
                """load dw result + GN + cond affine for expert e -> y2."""
                acc = work.tile([C, S], f32, tag="acc", name="acc")
                nc.sync.dma_start(out=acc, in_=y_scr[e][:, :])

                stats = small.tile([C, NCH, nc.vector.BN_STATS_DIM], f32,
                                   tag="stats", name="stats")
                for sc in range(NCH):
                    nc.vector.bn_stats(
                        out=stats[:, sc, :],
                        in_=acc[:, sc * NCHUNK:(sc + 1) * NCHUNK])
                mv = small.tile([C, nc.vector.BN_AGGR_DIM], f32, tag="mv",
                                name="mv")
                nc.vector.bn_aggr(out=mv, in_=stats)
                st3 = small.tile([C, 3], f32, tag="st3", name="st3")
                nc.vector.tensor_copy(st3[:, 0:2], mv)
                nc.vector.tensor_tensor(st3[:, 2:3], mv[:, 0:1], mv[:, 0:1],
                                        Alu.mult)
                ps_stat = ps_s.tile([1, 3], f32, tag="pstat", name="pstat",
                                    bufs=1)
                nc.tensor.matmul(ps_stat, ones96, st3, start=True, stop=True)

                st_sb = small.tile([1, 3], f32, tag="st_sb", name="st_sb")
                nc.vector.tensor_copy(st_sb, ps_stat)
                mean11 = small.tile([1, 1], f32, tag="mean11", name="mean11")
                nc.vector.tensor_scalar_mul(mean11, st_sb[0:1, 0:1], 1.0 / C)
                ex2 = small.tile([1, 1], f32, tag="ex2", name="ex2")
                nc.vector.tensor_tensor(ex2, st_sb[0:1, 1:2], st_sb[0:1, 2:3],
                                        Alu.add)
                var11 = small.tile([1, 1], f32, tag="var11", name="var11")
                nc.vector.tensor_scalar_mul(var11, ex2, 1.0 / C)
                msq11 = small.tile([1, 1], f32, tag="msq11", name="msq11")
                nc.vector.tensor_tensor(msq11, mean11, mean11, Alu.mult)
                nc.vector.tensor_tensor(var11, var11, msq11, Alu.subtract)
                std11 = small.tile([1, 1], f32, tag="std11", name="std11")
                nc.scalar.activation(std11, var11, Act.Sqrt, bias=eps11[0:1, 0:1])
                rstd11 = small.tile([1, 1], f32, tag="rstd11", name="rstd11")
                nc.vector.reciprocal(rstd11, std11)
                mr = small.tile([1, 2], f32, tag="mr", name="mr")
                nc.vector.tensor_copy(mr[:, 0:1], mean11)
                nc.vector.tensor_copy(mr[:, 1:2], rstd11)
                bc = ps_s.tile([C, 2], f32, tag="bc", name="bc", bufs=1)
                nc.tensor.matmul(bc, ones_row, mr, start=True, stop=True)

                a_vec = small.tile([C, 1], f32, tag="a_vec", name="a_vec")
                nc.vector.tensor_tensor(a_vec, gn_w[:, e:e + 1], bc[:, 1:2],
                                        Alu.mult)
                mb = small.tile([C, 1], f32, tag="mb", name="mb")
                nc.vector.tensor_tensor(mb, bc[:, 0:1], a_vec, Alu.mult)
                b_vec = small.tile([C, 1], f32, tag="b_vec", name="b_vec")
                nc.vector.tensor_tensor(b_vec, gn_b[:, e:e + 1], mb, Alu.subtract)

                y2 = y2p.tile([C + 1, S], f32, tag="y2", name="y2")
                nc.vector.memset(y2[C:C + 1, :], 1.0)
                nc.scalar.activation(y2[0:C, :], acc, Act.Identity,
                                     bias=b_vec, scale=a_vec)

                for sc in range(NCH):
                    sl = slice(sc * NCHUNK, (sc + 1) * NCHUNK)
                    pg = ps.tile([C, NCHUNK], f32, tag="mm", name="pg")
                    nc.tensor.matmul(pg, lhsT_c[:, e * C:(e + 1) * C],
                                     cond_sb[:, sl], start=True, stop=True)
                    gam = small.tile([C, NCHUNK], f32, tag="gam", name="gam")
                    nc.vector.tensor_scalar(gam, pg, cb_g[:, e:e + 1], 1.0,
                                            Alu.add, Alu.add)
                    pb = ps.tile([C, NCHUNK], f32, tag="mm", name="pb")
                    nc.tensor.matmul(pb, lhsT_c[:, EC + e * C:EC + (e + 1) * C],
                                     cond_sb[:, sl], start=True, stop=True)
                    bet = small.tile([C, NCHUNK], f32, tag="bet", name="bet")
                    nc.vector.tensor_scalar(bet, pb, cb_b[:, e:e + 1], None,
                                            Alu.add)
                    nc.vector.tensor_tensor(y2[0:C, sl], y2[0:C, sl], gam,
                                            Alu.mult)
                    nc.vector.tensor_tensor(y2[0:C, sl], y2[0:C, sl], bet,
                                            Alu.add)
                return y2

            ones_row = wt.tile([1, C], f32)
            nc.vector.memset(ones_row, 1.0)

            for b in range(E // 2):
                y2_lin = build_y2(b)
                y2_gate = build_y2(4 + b)
                for half in range(2):  # output experts 2b, 2b+1
                    e = 2 * b + half
                    y3 = work.tile([C, S], f32, tag="y3", bufs=2, name="y3")
                    for sc in range(NCH):
                        sl = slice(sc * NCHUNK, (sc + 1) * NCHUNK)
                        pl = ps.tile([C, NCHUNK], f32, tag="mm", name="pl")
                        nc.tensor.matmul(
                            pl, lhsT_in[:, b * 2 * C + half * C:
                                        b * 2 * C + (half + 1) * C],
                            y2_lin[:, sl], start=True, stop=True)
                        pgt = ps.tile([C, NCHUNK], f32, tag="mm", name="pgt")
                        nc.tensor.matmul(
                            pgt, lhsT_in[:, (4 + b) * 2 * C + half * C:
                                         (4 + b) * 2 * C + (half + 1) * C],
                            y2_gate[:, sl], start=True, stop=True)
                        sil = small.tile([C, NCHUNK], f32, tag="sil", name="sil")
                        nc.scalar.activation(sil, pl, Act.Silu)
                        gt = small.tile([C + 1, NCHUNK], f32, tag="gt", name="gt")
                        nc.vector.memset(gt[C:C + 1, :], 1.0)
                        nc.vector.tensor_tensor(gt[0:C, :], sil, pgt, Alu.mult)

                        po = ps.tile([C, NCHUNK], f32, tag="mm", name="po")
                        nc.tensor.matmul(po, lhsT_out[:, e * C:(e + 1) * C],
                                         gt, start=True, stop=True)
                        # residual add; writes y3 in (l,p,q) order via
                        # permuted out AP (chunk sc covers q in {2sc, 2sc+1})
                        qpc = NCHUNK // LP  # q's per chunk = 2
                        nc.vector.tensor_tensor(
                            y3.rearrange("c (l p q) -> c q l p", l=L, p=P)[
                                :, sc * qpc:(sc + 1) * qpc, :, :],
                            po.rearrange("c (q l p) -> c q l p", q=qpc, l=L),
                            x_sb[:, sl].rearrange("c (q l p) -> c q l p",
                                                  q=qpc, l=L),
                            Alu.add)
                    nc.sync.dma_start(out=out_d[e * C:(e + 1) * C, :], in_=y3)

    nc.finalize()
    return nc


def _get_built():
    global _BUILT
    if _BUILT is None:
        _BUILT = _build()
    return _BUILT


def _prep_static(inputs):
    """Host-side prep of weight-derived tensors (shared across cores)."""
    dw_w = np.asarray(inputs["dw_weight"], np.float32).reshape(EC, KS, KS)
    dw_b = np.asarray(inputs["dw_bias"], np.float32)
    band = np.zeros((KS, P, EC, P), np.float32)
    for i in range(KS):
        for dq in range(-PAD, PAD + 1):
            j = dq + PAD
            qo = np.arange(max(0, -dq), min(P, P - dq))
            band[i, qo + dq, :, qo] = dw_w[:, i, j][None, :]
    band = band.reshape(KS * P, EC * P)
    band = np.concatenate([band, np.tile(dw_b[:, None], (1, P)).reshape(1, EC * P)],
                          axis=0)
    return {
        "dw_band": np.ascontiguousarray(band),
        "gn_w": np.asarray(inputs["gn_weight"], np.float32),
        "gn_b": np.asarray(inputs["gn_bias"], np.float32),
        "pw_in_w": np.asarray(inputs["pw_in_weight"], np.float32),
        "pw_in_b": np.asarray(inputs["pw_in_bias"], np.float32),
        "pw_out_w": np.asarray(inputs["pw_out_weight"], np.float32),
        "pw_out_b": np.asarray(inputs["pw_out_bias"], np.float32),
        "cond_w": np.asarray(inputs["cond_w"], np.float32),
        "cond_b": np.asarray(inputs["cond_b"], np.float32),
    }


def _prep_core(x_k, cond_k):
    """Per-core prep: shifted-transposed rhs for dw, (q,l,p)-ordered x/cond."""
    xt = x_k.transpose(3, 0, 1, 2)  # [q, c, l, p]
    rhs = np.zeros((KS, P, C, L, P), np.float32)
    for i in range(KS):
        a, b = max(0, PAD - i), min(P, P + PAD - i)
        rhs[i, :, :, :, a:b] = xt[:, :, :, a + i - PAD:b + i - PAD]
    rhs = rhs.reshape(KS * P, C * LP)
    rhs = np.concatenate([rhs, np.ones((1, C * LP), np.float32)], axis=0)
    return {
        "dw_rhs": np.ascontiguousarray(rhs),
        "x_qlp": np.ascontiguousarray(
            x_k.transpose(0, 3, 1, 2).reshape(C, S)),
        "cond_qlp": np.ascontiguousarray(
            cond_k.transpose(0, 3, 1, 2).reshape(CONDC, S)),
    }


def kernel(**inputs):
    from concourse.bass_utils import run_bass_kernel_spmd

    nc = _get_built()
    x = np.asarray(inputs["x"], dtype=np.float32)
    cond = np.asarray(inputs["cond"], dtype=np.float32)
    base = _prep_static(inputs)
    in_maps = []
    for k in range(N):
        m = dict(base)
        m.update(_prep_core(x[k], cond[k]))
        in_maps.append(m)
    res = run_bass_kernel_spmd(nc, in_maps, list(range(N)))
    out = np.empty((N, E, C, L, P, P), dtype=np.float32)
    for k in range(N):
        out[k] = res.results[k]["out"].reshape(E, C, L, P, P)
    return out


# revision 16
# speedup vs baseline: 10157.5932x; 10157.5932x over previous
"""Trainium2 Bass kernel for nn_BatchedGatedConvExperts.

Data-parallel over N across 8 cores (core k handles batch n=k).

v2: depthwise 7x7 conv runs on the PE array as per-channel band-matrix
matmuls: contraction over (row-tap i, q_in) = 7*16=112 (+1 ones-row for
bias), rhs = host-prepared p-shifted copies of x^T, lhsT = host-prepared
band matrices w[e,c,i,qi-qo+3]. Output lands as [q_out, (l,p)] per channel,
packed 3 channels per [96,256] psum slab (PSUM write base must be 0/32/64),
staged via ACT to SBUF and DMA-flattened to a DRAM scratch in channel-major
[(e c), (q,l,p)] layout. Phase 2 (GroupNorm -> cond affine -> pw_in+SiLU
gate -> pw_out -> residual) runs in (q,l,p) spatial order; the final
residual add writes (l,p,q) order via a permuted output AP.

Flat-chunk quirk of the reference (torch .chunk on flat E*2C axis):
  silu input for output-expert e = pw_in block (e//2), rows (e%2)*96..+96,
  computed from y2 of expert e//2; gate half from block 4+e//2 / y2[4+e//2].
Experts processed as pairs b in 0..3: y2[b], y2[4+b] -> outputs 2b, 2b+1.
"""
import sys

sys.path.insert(0, "/opt/trn_rl_repo")

import numpy as np

E, C, KS, CONDC = 8, 96, 7, 32
N, L, P = 8, 16, 16
PAD = KS // 2
S = L * P * P  # 4096
EC = E * C  # 768
EPS = 1e-5
NCHUNK = 512
NCH = S // NCHUNK  # 8
KDW = KS * P + 1  # 113: (i, q_in) + ones row
LP = L * P  # 256

_BUILT = None


def _build():
    import concourse.bacc as bacc
    import concourse.mybir as mybir
    from concourse.masks import make_identity
    from concourse.tile import TileContext

    dt = mybir.dt
    f32 = dt.float32
    Alu = mybir.AluOpType
    Act = mybir.ActivationFunctionType

    nc = bacc.Bacc(None, target_bir_lowering=False)

    xq_d = nc.declare_dram_parameter("x_qlp", [C, S], f32, isOutput=False)
    condq_d = nc.declare_dram_parameter("cond_qlp", [CONDC, S], f32, isOutput=False)
    rhs_d = nc.declare_dram_parameter("dw_rhs", [KDW, C * LP], f32, isOutput=False)
    band_d = nc.declare_dram_parameter("dw_band", [KDW, EC * P], f32, isOutput=False)
    gnw_d = nc.declare_dram_parameter("gn_w", [EC], f32, isOutput=False)
    gnb_d = nc.declare_dram_parameter("gn_b", [EC], f32, isOutput=False)
    piw_d = nc.declare_dram_parameter("pw_in_w", [2 * EC, C], f32, isOutput=False)
    pib_d = nc.declare_dram_parameter("pw_in_b", [2 * EC], f32, isOutput=False)
    pow_d = nc.declare_dram_parameter("pw_out_w", [EC, C], f32, isOutput=False)
    pob_d = nc.declare_dram_parameter("pw_out_b", [EC], f32, isOutput=False)
    cw_d = nc.declare_dram_parameter("cond_w", [2 * EC, CONDC], f32, isOutput=False)
    cb_d = nc.declare_dram_parameter("cond_b", [2 * EC], f32, isOutput=False)
    out_d = nc.declare_dram_parameter("out", [EC, S], f32, isOutput=True)

    with TileContext(nc) as tc:
        # per-expert DRAM scratch for dw output, [(c), (q,l,p)]
        dram_cm = tc.tile_pool(name="dram", bufs=1, space="DRAM")
        dram = dram_cm.__enter__()
        y_scr = [dram.tile([C, S], f32, name=f"y_scr{e}", tag=f"yscr{e}")
                 for e in range(E)]

        # ---------------- phase 1: depthwise conv on PE ----------------
        with tc.tile_pool(name="p1", bufs=1) as p1, \
             tc.tile_pool(name="p1w", bufs=4) as p1w, \
             tc.tile_pool(name="ps1", bufs=4, space="PSUM") as ps1:
            rhs = p1.tile([KDW, C * LP], f32)
            H = C * LP // 2
            nc.sync.dma_start(out=rhs[:, :H], in_=rhs_d[:, :H])
            nc.scalar.dma_start(out=rhs[:, H:], in_=rhs_d[:, H:])
            band = p1.tile([KDW, EC * P], f32)
            HB = EC * P // 2
            nc.sync.dma_start(out=band[:, :HB], in_=band_d[:, :HB])
            nc.scalar.dma_start(out=band[:, HB:], in_=band_d[:, HB:])

            for t3 in range(EC // 3):  # 256 slabs of 3 channels
                pslab = ps1.tile([96, LP], f32, tag="dwps", name="pslab")
                for k in range(3):
                    fc = 3 * t3 + k  # flat (e,c) channel
                    c = fc % C
                    nc.tensor.matmul(
                        pslab[32 * k:32 * k + P, :],
                        band[:, fc * P:(fc + 1) * P],
                        rhs[:, c * LP:(c + 1) * LP],
                        start=True, stop=True)
                stg = p1w.tile([96, LP], f32, tag="stg", name="stg")
                if t3 % 2 == 0:
                    nc.scalar.copy(stg, pslab)
                else:
                    nc.vector.tensor_copy(stg, pslab)
                for k in range(3):
                    fc = 3 * t3 + k
                    e, c = fc // C, fc % C
                    dma_eng = nc.sync if (t3 % 2 == 0) else nc.scalar
                    dma_eng.dma_start(
                        out=y_scr[e][c:c + 1, :].rearrange("o (q lp) -> o q lp", q=P),
                        in_=stg[32 * k:32 * k + P, :])

        # ---------------- phase 2 ----------------
        with tc.tile_pool(name="wt", bufs=1) as wt, \
             tc.tile_pool(name="big", bufs=1) as big, \
             tc.tile_pool(name="y2p", bufs=3) as y2p, \
             tc.tile_pool(name="work", bufs=2) as work, \
             tc.tile_pool(name="small", bufs=4) as small, \
             tc.tile_pool(name="ps", bufs=4, space="PSUM") as ps, \
             tc.tile_pool(name="ps_s", bufs=1, space="PSUM") as ps_s:

            ident = wt.tile([128, 128], f32)
            make_identity(nc, ident)

            gn_w = wt.tile([C, E], f32)
            nc.sync.dma_start(out=gn_w, in_=gnw_d[:].rearrange("(e c) -> c e", e=E))
            gn_b = wt.tile([C, E], f32)
            nc.sync.dma_start(out=gn_b, in_=gnb_d[:].rearrange("(e c) -> c e", e=E))
            cb_g = wt.tile([C, E], f32)
            nc.sync.dma_start(out=cb_g, in_=cb_d[:EC].rearrange("(e c) -> c e", e=E))
            cb_b = wt.tile([C, E], f32)
            nc.sync.dma_start(out=cb_b, in_=cb_d[EC:].rearrange("(e c) -> c e", e=E))

            lhsT_in = wt.tile([C + 1, 2 * EC], f32)
            lhsT_out = wt.tile([C + 1, EC], f32)
            lhsT_c = wt.tile([CONDC, 2 * EC], f32)
            for t in range(2 * EC // 128):
                w_raw = work.tile([128, C], f32, tag="wraw", name="wraw")
                nc.sync.dma_start(out=w_raw, in_=piw_d[t * 128:(t + 1) * 128, :])
                pt = ps.tile([C, 128], f32, tag="wtr", bufs=2, name="pt")
                nc.tensor.transpose(pt, w_raw, ident)
                nc.vector.tensor_copy(lhsT_in[0:C, t * 128:(t + 1) * 128], pt)
            for t in range(EC // 128):
                w_raw = work.tile([128, C], f32, tag="wraw", name="wraw")
                nc.sync.dma_start(out=w_raw, in_=pow_d[t * 128:(t + 1) * 128, :])
                pt = ps.tile([C, 128], f32, tag="wtr", bufs=2, name="pt")
                nc.tensor.transpose(pt, w_raw, ident)
                nc.vector.tensor_copy(lhsT_out[0:C, t * 128:(t + 1) * 128], pt)
            for t in range(2 * EC // 128):
                w_raw = work.tile([128, CONDC], f32, tag="wraw", name="wraw")
                nc.sync.dma_start(out=w_raw, in_=cw_d[t * 128:(t + 1) * 128, :])
                pt = ps.tile([CONDC, 128], f32, tag="wtr", bufs=2, name="pt")
                nc.tensor.transpose(pt, w_raw, ident)
                nc.vector.tensor_copy(lhsT_c[:, t * 128:(t + 1) * 128], pt)
            nc.sync.dma_start(out=lhsT_in[C:C + 1, :], in_=pib_d[:])
            nc.sync.dma_start(out=lhsT_out[C:C + 1, :], in_=pob_d[:])

            cond_sb = big.tile([CONDC, S], f32)
            nc.sync.dma_start(out=cond_sb, in_=condq_d[:])
            x_sb = big.tile([C, S], f32)
            nc.sync.dma_start(out=x_sb, in_=xq_d[:])

            ones96 = wt.tile([C, 1], f32)
            nc.vector.memset(ones96, 1.0)
            eps11 = wt.tile([1, 1], f32)
            nc.vector.memset(eps11, EPS)

            def build_y2(e):
                """load dw result + GN + cond affine for expert e -> y2."""
                acc = work.tile([C, S], f32, tag="acc", name="acc")
                nc.scalar.dma_start(out=acc, in_=y_scr[e][:, :])

                stats = small.tile([C, NCH, nc.vector.BN_STATS_DIM], f32,
                                   tag="stats", name="stats")
                for sc in range(NCH):
                    nc.vector.bn_stats(
                        out=stats[:, sc, :],
                        in_=acc[:, sc * NCHUNK:(sc + 1) * NCHUNK])
                mv = small.tile([C, nc.vector.BN_AGGR_DIM], f32, tag="mv",
                                name="mv")
                nc.vector.bn_aggr(out=mv, in_=stats)
                st3 = small.tile([C, 3], f32, tag="st3", name="st3")
                nc.vector.tensor_copy(st3[:, 0:2], mv)
                nc.vector.tensor_tensor(st3[:, 2:3], mv[:, 0:1], mv[:, 0:1],
                                        Alu.mult)
                ps_stat = ps_s.tile([1, 3], f32, tag="pstat", name="pstat",
                                    bufs=1)
                nc.tensor.matmul(ps_stat, ones96, st3, start=True, stop=True)

                st_sb = small.tile([1, 3], f32, tag="st_sb", name="st_sb")
                nc.vector.tensor_copy(st_sb, ps_stat)
                mean11 = small.tile([1, 1], f32, tag="mean11", name="mean11")
                nc.vector.tensor_scalar_mul(mean11, st_sb[0:1, 0:1], 1.0 / C)
                ex2 = small.tile([1, 1], f32, tag="ex2", name="ex2")
                nc.vector.tensor_tensor(ex2, st_sb[0:1, 1:2], st_sb[0:1, 2:3],
                                        Alu.add)
                var11 = small.tile([1, 1], f32, tag="var11", name="var11")
                nc.vector.tensor_scalar_mul(var11, ex2, 1.0 / C)
                msq11 = small.tile([1, 1], f32, tag="msq11", name="msq11")
                nc.vector.tensor_tensor(msq11, mean11, mean11, Alu.mult)
                nc.vector.tensor_tensor(var11, var11, msq11, Alu.subtract)
                std11 = small.tile([1, 1], f32, tag="std11", name="std11")
                nc.scalar.activation(std11, var11, Act.Sqrt, bias=eps11[0:1, 0:1])
                rstd11 = small.tile([1, 1], f32, tag="rstd11", name="rstd11")
                nc.vector.reciprocal(rstd11, std11)
                mr = small.tile([1, 2], f32, tag="mr", name="mr")
                nc.vector.tensor_copy(mr[:, 0:1], mean11)
                nc.vector.tensor_copy(mr[:, 1:2], rstd11)
                bc = ps_s.tile([C, 2], f32, tag="bc", name="bc", bufs=1)
                nc.tensor.matmul(bc, ones_row, mr, start=True, stop=True)

                a_vec = small.tile([C, 1], f32, tag="a_vec", name="a_vec")
                nc.vector.tensor_tensor(a_vec, gn_w[:, e:e + 1], bc[:, 1:2],
                                        Alu.mult)
                mb = small.tile([C, 1], f32, tag="mb", name="mb")
                nc.vector.tensor_tensor(mb, bc[:, 0:1], a_vec, Alu.mult)
                b_vec = small.tile([C, 1], f32, tag="b_vec", name="b_vec")
                nc.vector.tensor_tensor(b_vec, gn_b[:, e:e + 1], mb, Alu.subtract)

                y2 = y2p.tile([C + 1, S], f32, tag="y2", name="y2")
                nc.vector.memset(y2[C:C + 1, :], 1.0)
                nc.scalar.activation(y2[0:C, :], acc, Act.Identity,
                                     bias=b_vec, scale=a_vec)

                for sc in range(NCH):
                    sl = slice(sc * NCHUNK, (sc + 1) * NCHUNK)
                    pg = ps.tile([C, NCHUNK], f32, tag="mm", name="pg")
                    nc.tensor.matmul(pg, lhsT_c[:, e * C:(e + 1) * C],
                                     cond_sb[:, sl], start=True, stop=True)
                    gam = small.tile([C, NCHUNK], f32, tag="gam", name="gam")
                    nc.vector.tensor_scalar(gam, pg, cb_g[:, e:e + 1], 1.0,
                                            Alu.add, Alu.add)
                    pb = ps.tile([C, NCHUNK], f32, tag="mm", name="pb")
                    nc.tensor.matmul(pb, lhsT_c[:, EC + e * C:EC + (e + 1) * C],
                                     cond_sb[:, sl], start=True, stop=True)
                    bet = small.tile([C, NCHUNK], f32, tag="bet", name="bet")
                    nc.vector.tensor_scalar(bet, pb, cb_b[:, e:e + 1], None,
                                            Alu.add)
                    nc.vector.tensor_tensor(y2[0:C, sl], y2[0:C, sl], gam,
                                            Alu.mult)
                    nc.gpsimd.tensor_tensor(y2[0:C, sl], y2[0:C, sl], bet,
                                            Alu.add)
                return y2

            ones_row = wt.tile([1, C], f32)
            nc.vector.memset(ones_row, 1.0)

            for b in range(E // 2):
                y2_lin = build_y2(b)
                y2_gate = build_y2(4 + b)
                for half in range(2):  # output experts 2b, 2b+1
                    e = 2 * b + half
                    y3 = work.tile([C, S], f32, tag="y3", bufs=2, name="y3")
                    gt = work.tile([C + 1, S], f32, tag="gt", bufs=1, name="gt")
                    nc.vector.memset(gt[C:C + 1, :], 1.0)
                    for sc in range(NCH):
                        sl = slice(sc * NCHUNK, (sc + 1) * NCHUNK)
                        pl = ps.tile([C, NCHUNK], f32, tag="mm", name="pl")
                        nc.tensor.matmul(
                            pl, lhsT_in[:, b * 2 * C + half * C:
                                        b * 2 * C + (half + 1) * C],
                            y2_lin[:, sl], start=True, stop=True)
                        pgt = ps.tile([C, NCHUNK], f32, tag="mm", name="pgt")
                        nc.tensor.matmul(
                            pgt, lhsT_in[:, (4 + b) * 2 * C + half * C:
                                         (4 + b) * 2 * C + (half + 1) * C],
                            y2_gate[:, sl], start=True, stop=True)
                        sil = small.tile([C, NCHUNK], f32, tag="sil", name="sil")
                        nc.scalar.activation(sil, pl, Act.Silu)
                        nc.vector.tensor_tensor(gt[0:C, sl], sil, pgt, Alu.mult)

                        po = ps.tile([C, NCHUNK], f32, tag="mm", name="po")
                        nc.tensor.matmul(po, lhsT_out[:, e * C:(e + 1) * C],
                                         gt[:, sl], start=True, stop=True)
                        # residual add; writes y3 in (l,p,q) order via
                        # permuted out AP (chunk sc covers q in {2sc, 2sc+1})
                        qpc = NCHUNK // LP  # q's per chunk = 2
                        nc.vector.tensor_tensor(
                            y3.rearrange("c (l p q) -> c q l p", l=L, p=P)[
                                :, sc * qpc:(sc + 1) * qpc, :, :],
                            po.rearrange("c (q l p) -> c q l p", q=qpc, l=L),
                            x_sb[:, sl].rearrange("c (q l p) -> c q l p",
                                                  q=qpc, l=L),
                            Alu.add)
                    nc.scalar.dma_start(out=out_d[e * C:(e + 1) * C, :], in_=y3)

        dram_cm.__exit__(None, None, None)

    nc.finalize()
    return nc


def _get_built():
    global _BUILT
    if _BUILT is None:
        _BUILT = _build()
    return _BUILT


def _prep_static(inputs):
    """Host-side prep of weight-derived tensors (shared across cores)."""
    dw_w = np.asarray(inputs["dw_weight"], np.float32).reshape(EC, KS, KS)
    dw_b = np.asarray(inputs["dw_bias"], np.float32)
    band = np.zeros((KS, P, EC, P), np.float32)
    for i in range(KS):
        for dq in range(-PAD, PAD + 1):
            j = dq + PAD
            qo = np.arange(max(0, -dq), min(P, P - dq))
            band[i, qo + dq, :, qo] = dw_w[:, i, j][None, :]
    band = band.reshape(KS * P, EC * P)
    band = np.concatenate([band, np.tile(dw_b[:, None], (1, P)).reshape(1, EC * P)],
                          axis=0)
    return {
        "dw_band": np.ascontiguousarray(band),
        "gn_w": np.asarray(inputs["gn_weight"], np.float32),
        "gn_b": np.asarray(inputs["gn_bias"], np.float32),
        "pw_in_w": np.asarray(inputs["pw_in_weight"], np.float32),
        "pw_in_b": np.asarray(inputs["pw_in_bias"], np.float32),
        "pw_out_w": np.asarray(inputs["pw_out_weight"], np.float32),
        "pw_out_b": np.asarray(inputs["pw_out_bias"], np.float32),
        "cond_w": np.asarray(inputs["cond_w"], np.float32),
        "cond_b": np.asarray(inputs["cond_b"], np.float32),
    }


def _prep_core(x_k, cond_k):
    """Per-core prep: shifted-transposed rhs for dw, (q,l,p)-ordered x/cond."""
    xt = x_k.transpose(3, 0, 1, 2)  # [q, c, l, p]
    rhs = np.zeros((KS, P, C, L, P), np.float32)
    for i in range(KS):
        a, b = max(0, PAD - i), min(P, P + PAD - i)
        rhs[i, :, :, :, a:b] = xt[:, :, :, a + i - PAD:b + i - PAD]
    rhs = rhs.reshape(KS * P, C * LP)
    rhs = np.concatenate([rhs, np.ones((1, C * LP), np.float32)], axis=0)
    return {
        "dw_rhs": np.ascontiguousarray(rhs),
        "x_qlp": np.ascontiguousarray(
            x_k.transpose(0, 3, 1, 2).reshape(C, S)),
        "cond_qlp": np.ascontiguousarray(
            cond_k.transpose(0, 3, 1, 2).reshape(CONDC, S)),
    }


def kernel(**inputs):
    from concourse.bass_utils import run_bass_kernel_spmd

    nc = _get_built()
    x = np.asarray(inputs["x"], dtype=np.float32)
    cond = np.asarray(inputs["cond"], dtype=np.float32)
    base = _prep_static(inputs)
    in_maps = []
    for k in range(N):
        m = dict(base)
        m.update(_prep_core(x[k], cond[k]))
        in_maps.append(m)
    res = run_bass_kernel_spmd(nc, in_maps, list(range(N)))
    out = np.empty((N, E, C, L, P, P), dtype=np.float32)
    for k in range(N):
        out[k] = res.results[k]["out"].reshape(E, C, L, P, P)
    return out


# revision 21
# speedup vs baseline: 10272.6617x; 1.0113x over previous
"""Trainium2 Bass kernel for nn_BatchedGatedConvExperts.

Data-parallel over N across 8 cores (core k handles batch n=k).

v2: depthwise 7x7 conv runs on the PE array as per-channel band-matrix
matmuls: contraction over (row-tap i, q_in) = 7*16=112 (+1 ones-row for
bias), rhs = host-prepared p-shifted copies of x^T, lhsT = host-prepared
band matrices w[e,c,i,qi-qo+3]. Output lands as [q_out, (l,p)] per channel,
packed 3 channels per [96,256] psum slab (PSUM write base must be 0/32/64),
staged via ACT to SBUF and DMA-flattened to a DRAM scratch in channel-major
[(e c), (q,l,p)] layout. Phase 2 (GroupNorm -> cond affine -> pw_in+SiLU
gate -> pw_out -> residual) runs in (q,l,p) spatial order; the final
residual add writes (l,p,q) order via a permuted output AP.

Flat-chunk quirk of the reference (torch .chunk on flat E*2C axis):
  silu input for output-expert e = pw_in block (e//2), rows (e%2)*96..+96,
  computed from y2 of expert e//2; gate half from block 4+e//2 / y2[4+e//2].
Experts processed as pairs b in 0..3: y2[b], y2[4+b] -> outputs 2b, 2b+1.
"""
import sys

sys.path.insert(0, "/opt/trn_rl_repo")

import numpy as np

E, C, KS, CONDC = 8, 96, 7, 32
N, L, P = 8, 16, 16
PAD = KS // 2
S = L * P * P  # 4096
EC = E * C  # 768
EPS = 1e-5
NCHUNK = 512
NCH = S // NCHUNK  # 8
KDW = KS * P + 1  # 113: (i, q_in) + ones row
LP = L * P  # 256

_BUILT = None


def _build():
    import concourse.bacc as bacc
    import concourse.mybir as mybir
    from concourse.masks import make_identity
    from concourse.tile import TileContext

    dt = mybir.dt
    f32 = dt.float32
    Alu = mybir.AluOpType
    Act = mybir.ActivationFunctionType

    nc = bacc.Bacc(None, target_bir_lowering=False)

    xq_d = nc.declare_dram_parameter("x_qlp", [C, S], f32, isOutput=False)
    condq_d = nc.declare_dram_parameter("cond_qlp", [CONDC, S], f32, isOutput=False)
    rhs_d = nc.declare_dram_parameter("dw_rhs", [KDW, C * LP], f32, isOutput=False)
    band_d = nc.declare_dram_parameter("dw_band", [KDW, EC * P], f32, isOutput=False)
    gnw_d = nc.declare_dram_parameter("gn_w", [EC], f32, isOutput=False)
    gnb_d = nc.declare_dram_parameter("gn_b", [EC], f32, isOutput=False)
    piw_d = nc.declare_dram_parameter("pw_in_w", [2 * EC, C], f32, isOutput=False)
    pib_d = nc.declare_dram_parameter("pw_in_b", [2 * EC], f32, isOutput=False)
    pow_d = nc.declare_dram_parameter("pw_out_w", [EC, C], f32, isOutput=False)
    pob_d = nc.declare_dram_parameter("pw_out_b", [EC], f32, isOutput=False)
    cw_d = nc.declare_dram_parameter("cond_w", [2 * EC, CONDC], f32, isOutput=False)
    cb_d = nc.declare_dram_parameter("cond_b", [2 * EC], f32, isOutput=False)
    out_d = nc.declare_dram_parameter("out", [EC, S], f32, isOutput=True)

    with TileContext(nc) as tc:
        # per-expert DRAM scratch for dw output, [(c), (q,l,p)]
        dram_cm = tc.tile_pool(name="dram", bufs=1, space="DRAM")
        dram = dram_cm.__enter__()
        y_scr = [dram.tile([C, S], f32, name=f"y_scr{e}", tag=f"yscr{e}")
                 for e in range(E)]

        # ---------------- weight prep (overlaps phase-1 input DMA) ----------
        wt_cm = tc.tile_pool(name="wt", bufs=1)
        wt = wt_cm.__enter__()
        with tc.tile_pool(name="wprep", bufs=2) as wprep, \
             tc.tile_pool(name="ps_w", bufs=2, space="PSUM") as ps_w:
            ident = wt.tile([128, 128], f32)
            make_identity(nc, ident)

            gn_w = wt.tile([C, E], f32)
            nc.sync.dma_start(out=gn_w, in_=gnw_d[:].rearrange("(e c) -> c e", e=E))
            gn_b = wt.tile([C, E], f32)
            nc.sync.dma_start(out=gn_b, in_=gnb_d[:].rearrange("(e c) -> c e", e=E))
            cb_g = wt.tile([C, E], f32)
            nc.sync.dma_start(out=cb_g, in_=cb_d[:EC].rearrange("(e c) -> c e", e=E))
            cb_b = wt.tile([C, E], f32)
            nc.sync.dma_start(out=cb_b, in_=cb_d[EC:].rearrange("(e c) -> c e", e=E))
            cbg1 = wt.tile([C, E], f32)
            nc.vector.tensor_scalar_add(cbg1, cb_g, 1.0)

            lhsT_in = wt.tile([C + 1, 2 * EC], f32)
            lhsT_out = wt.tile([C + 1, EC], f32)
            lhsT_c = wt.tile([CONDC, 2 * EC], f32)
            for t in range(2 * EC // 128):
                w_raw = wprep.tile([128, C], f32, tag="wraw", name="wraw")
                nc.sync.dma_start(out=w_raw, in_=piw_d[t * 128:(t + 1) * 128, :])
                pt = ps_w.tile([C, 128], f32, tag="wtr", name="pt")
                nc.tensor.transpose(pt, w_raw, ident)
                nc.vector.tensor_copy(lhsT_in[0:C, t * 128:(t + 1) * 128], pt)
            for t in range(EC // 128):
                w_raw = wprep.tile([128, C], f32, tag="wraw", name="wraw")
                nc.sync.dma_start(out=w_raw, in_=pow_d[t * 128:(t + 1) * 128, :])
                pt = ps_w.tile([C, 128], f32, tag="wtr", name="pt")
                nc.tensor.transpose(pt, w_raw, ident)
                nc.vector.tensor_copy(lhsT_out[0:C, t * 128:(t + 1) * 128], pt)
            for t in range(2 * EC // 128):
                w_raw = wprep.tile([128, CONDC], f32, tag="wraw", name="wraw")
                nc.sync.dma_start(out=w_raw, in_=cw_d[t * 128:(t + 1) * 128, :])
                pt = ps_w.tile([CONDC, 128], f32, tag="wtr", name="pt")
                nc.tensor.transpose(pt, w_raw, ident)
                nc.vector.tensor_copy(lhsT_c[:, t * 128:(t + 1) * 128], pt)
            nc.sync.dma_start(out=lhsT_in[C:C + 1, :], in_=pib_d[:])
            nc.sync.dma_start(out=lhsT_out[C:C + 1, :], in_=pob_d[:])

            ones96 = wt.tile([C, 1], f32)
            nc.vector.memset(ones96, 1.0)
            ones_row = wt.tile([1, C], f32)
            nc.vector.memset(ones_row, 1.0)
            eps11 = wt.tile([1, 1], f32)
            nc.vector.memset(eps11, EPS)

        # ---------------- phase 1: depthwise conv on PE ----------------
        with tc.tile_pool(name="p1", bufs=1) as p1, \
             tc.tile_pool(name="p1w", bufs=4) as p1w, \
             tc.tile_pool(name="ps1", bufs=4, space="PSUM") as ps1:
            # rhs/band as per-quarter tiles so early channels' matmuls can
            # start while later quarters are still loading
            NQ = 4
            rhs_t, band_t = [], []
            for qd in range(NQ):
                rq = p1.tile([KDW, C * LP // NQ], f32, name=f"rhs{qd}",
                             tag=f"rhs{qd}")
                eng = nc.sync if qd % 2 == 0 else nc.scalar
                eng.dma_start(out=rq, in_=rhs_d[:, qd * C * LP // NQ:
                                               (qd + 1) * C * LP // NQ])
                rhs_t.append(rq)
                bq = p1.tile([KDW, EC * P // NQ], f32, name=f"band{qd}",
                             tag=f"band{qd}")
                eng2 = nc.scalar if qd % 2 == 0 else nc.sync
                eng2.dma_start(out=bq, in_=band_d[:, qd * EC * P // NQ:
                                                  (qd + 1) * EC * P // NQ])
                band_t.append(bq)
            CQ = C // NQ    # channels per rhs quarter (24)
            FQ = EC // NQ   # flat channels per band quarter (192)

            # slab order: rhs-quarter-major so early matmuls only need
            # quarter 0 of rhs; band is host-reordered to match (see
            # _prep_static), so band quarter qd holds its slabs contiguously.
            slab_channels = []
            for qd in range(NQ):
                for e in range(E):
                    for t in range(CQ // 3):
                        slab_channels.append(
                            [e * C + qd * CQ + 3 * t + k for k in range(3)])
            for t3, fcs in enumerate(slab_channels):
                pslab = ps1.tile([96, LP], f32, tag="dwps", name="pslab")
                for k, fc in enumerate(fcs):
                    c = fc % C
                    qd = c // CQ
                    bo = (t3 % (len(slab_channels) // NQ)) * 3 + k
                    rq, ro = rhs_t[qd], c % CQ
                    nc.tensor.matmul(
                        pslab[32 * k:32 * k + P, :],
                        band_t[qd][:, bo * P:(bo + 1) * P],
                        rq[:, ro * LP:(ro + 1) * LP],
                        start=True, stop=True)
                stg = p1w.tile([96, LP], f32, tag="stg", name="stg")
                if t3 % 2 == 0:
                    nc.scalar.copy(stg, pslab)
                else:
                    nc.vector.tensor_copy(stg, pslab)
                for k, fc in enumerate(fcs):
                    e, c = fc // C, fc % C
                    dma_eng = nc.sync if (t3 % 2 == 0) else nc.scalar
                    dma_eng.dma_start(
                        out=y_scr[e][c:c + 1, :].rearrange("o (q lp) -> o q lp", q=P),
                        in_=stg[32 * k:32 * k + P, :])

        # ---------------- phase 2 ----------------
        with tc.tile_pool(name="big", bufs=1) as big, \
             tc.tile_pool(name="y2p", bufs=3) as y2p, \
             tc.tile_pool(name="work", bufs=2) as work, \
             tc.tile_pool(name="small", bufs=4) as small, \
             tc.tile_pool(name="ps", bufs=4, space="PSUM") as ps, \
             tc.tile_pool(name="ps_s", bufs=1, space="PSUM") as ps_s:

            cond_sb = big.tile([CONDC, S], f32)
            nc.sync.dma_start(out=cond_sb, in_=condq_d[:])
            x_sb = big.tile([C, S], f32)
            nc.sync.dma_start(out=x_sb, in_=xq_d[:])


            def build_y2(e):
                """load dw result + GN + cond affine for expert e -> y2."""
                acc = work.tile([C, S], f32, tag="acc", name="acc")
                nc.scalar.dma_start(out=acc, in_=y_scr[e][:, :])

                stats = small.tile([C, NCH, nc.vector.BN_STATS_DIM], f32,
                                   tag="stats", name="stats")
                for sc in range(NCH):
                    nc.vector.bn_stats(
                        out=stats[:, sc, :],
                        in_=acc[:, sc * NCHUNK:(sc + 1) * NCHUNK])
                mv = small.tile([C, nc.vector.BN_AGGR_DIM], f32, tag="mv",
                                name="mv")
                nc.vector.bn_aggr(out=mv, in_=stats)
                st3 = small.tile([C, 3], f32, tag="st3", name="st3")
                nc.vector.tensor_copy(st3[:, 0:2], mv)
                nc.vector.tensor_tensor(st3[:, 2:3], mv[:, 0:1], mv[:, 0:1],
                                        Alu.mult)
                ps_stat = ps_s.tile([1, 3], f32, tag="pstat", name="pstat",
                                    bufs=1)
                nc.tensor.matmul(ps_stat, ones96, st3, start=True, stop=True)

                st_sb = small.tile([1, 3], f32, tag="st_sb", name="st_sb")
                nc.vector.tensor_copy(st_sb, ps_stat)
                mean11 = small.tile([1, 1], f32, tag="mean11", name="mean11")
                nc.vector.tensor_scalar_mul(mean11, st_sb[0:1, 0:1], 1.0 / C)
                ex2 = small.tile([1, 1], f32, tag="ex2", name="ex2")
                nc.vector.tensor_tensor(ex2, st_sb[0:1, 1:2], st_sb[0:1, 2:3],
                                        Alu.add)
                var11 = small.tile([1, 1], f32, tag="var11", name="var11")
                nc.vector.tensor_scalar_mul(var11, ex2, 1.0 / C)
                msq11 = small.tile([1, 1], f32, tag="msq11", name="msq11")
                nc.vector.tensor_tensor(msq11, mean11, mean11, Alu.mult)
                nc.vector.tensor_tensor(var11, var11, msq11, Alu.subtract)
                std11 = small.tile([1, 1], f32, tag="std11", name="std11")
                nc.scalar.activation(std11, var11, Act.Sqrt, bias=eps11[0:1, 0:1])
                rstd11 = small.tile([1, 1], f32, tag="rstd11", name="rstd11")
                nc.vector.reciprocal(rstd11, std11)
                mr = small.tile([1, 2], f32, tag="mr", name="mr")
                nc.vector.tensor_copy(mr[:, 0:1], mean11)
                nc.vector.tensor_copy(mr[:, 1:2], rstd11)
                bc = ps_s.tile([C, 2], f32, tag="bc", name="bc", bufs=1)
                nc.tensor.matmul(bc, ones_row, mr, start=True, stop=True)

                a_vec = small.tile([C, 1], f32, tag="a_vec", name="a_vec")
                nc.vector.tensor_tensor(a_vec, gn_w[:, e:e + 1], bc[:, 1:2],
                                        Alu.mult)
                mb = small.tile([C, 1], f32, tag="mb", name="mb")
                nc.vector.tensor_tensor(mb, bc[:, 0:1], a_vec, Alu.mult)
                b_vec = small.tile([C, 1], f32, tag="b_vec", name="b_vec")
                nc.vector.tensor_tensor(b_vec, gn_b[:, e:e + 1], mb, Alu.subtract)

                y2 = y2p.tile([C + 1, S], f32, tag="y2", name="y2")
                nc.vector.memset(y2[C:C + 1, :], 1.0)
                nc.scalar.activation(y2[0:C, :], acc, Act.Identity,
                                     bias=b_vec, scale=a_vec)

                for sc in range(NCH):
                    sl = slice(sc * NCHUNK, (sc + 1) * NCHUNK)
                    pg = ps.tile([C, NCHUNK], f32, tag="mm", name="pg")
                    nc.tensor.matmul(pg, lhsT_c[:, e * C:(e + 1) * C],
                                     cond_sb[:, sl], start=True, stop=True)
                    gam = small.tile([C, NCHUNK], f32, tag="gam", name="gam")
                    nc.vector.tensor_scalar(gam, pg, cbg1[:, e:e + 1], None,
                                            Alu.add)
                    pb = ps.tile([C, NCHUNK], f32, tag="mm", name="pb")
                    nc.tensor.matmul(pb, lhsT_c[:, EC + e * C:EC + (e + 1) * C],
                                     cond_sb[:, sl], start=True, stop=True)
                    bet = small.tile([C, NCHUNK], f32, tag="bet", name="bet")
                    nc.vector.tensor_scalar(bet, pb, cb_b[:, e:e + 1], None,
                                            Alu.add)
                    nc.gpsimd.tensor_tensor(y2[0:C, sl], y2[0:C, sl], gam,
                                            Alu.mult)
                    nc.gpsimd.tensor_tensor(y2[0:C, sl], y2[0:C, sl], bet,
                                            Alu.add)
                return y2


            for b in range(E // 2):
                y2_lin = build_y2(b)
                y2_gate = build_y2(4 + b)
                for half in range(2):  # output experts 2b, 2b+1
                    e = 2 * b + half
                    y3 = work.tile([C, S], f32, tag="y3", bufs=2, name="y3")
                    gt = work.tile([C + 1, S], f32, tag="gt", bufs=1, name="gt")
                    nc.vector.memset(gt[C:C + 1, :], 1.0)
                    for sc in range(NCH):
                        sl = slice(sc * NCHUNK, (sc + 1) * NCHUNK)
                        pl = ps.tile([C, NCHUNK], f32, tag="mm", name="pl")
                        nc.tensor.matmul(
                            pl, lhsT_in[:, b * 2 * C + half * C:
                                        b * 2 * C + (half + 1) * C],
                            y2_lin[:, sl], start=True, stop=True)
                        pgt = ps.tile([C, NCHUNK], f32, tag="mm", name="pgt")
                        nc.tensor.matmul(
                            pgt, lhsT_in[:, (4 + b) * 2 * C + half * C:
                                         (4 + b) * 2 * C + (half + 1) * C],
                            y2_gate[:, sl], start=True, stop=True)
                        sil = small.tile([C, NCHUNK], f32, tag="sil", name="sil")
                        nc.scalar.activation(sil, pl, Act.Silu)
                        nc.vector.tensor_tensor(gt[0:C, sl], sil, pgt, Alu.mult)

                        po = ps.tile([C, NCHUNK], f32, tag="mm", name="po")
                        nc.tensor.matmul(po, lhsT_out[:, e * C:(e + 1) * C],
                                         gt[:, sl], start=True, stop=True)
                        # residual add; writes y3 in (l,p,q) order via
                        # permuted out AP (chunk sc covers q in {2sc, 2sc+1})
                        qpc = NCHUNK // LP  # q's per chunk = 2
                        nc.vector.tensor_tensor(
                            y3.rearrange("c (l p q) -> c q l p", l=L, p=P)[
                                :, sc * qpc:(sc + 1) * qpc, :, :],
                            po.rearrange("c (q l p) -> c q l p", q=qpc, l=L),
                            x_sb[:, sl].rearrange("c (q l p) -> c q l p",
                                                  q=qpc, l=L),
                            Alu.add)
                    nc.scalar.dma_start(out=out_d[e * C:(e + 1) * C, :], in_=y3)

        wt_cm.__exit__(None, None, None)
        dram_cm.__exit__(None, None, None)

    nc.finalize()
    return nc


def _get_built():
    global _BUILT
    if _BUILT is None:
        _BUILT = _build()
    return _BUILT


def _prep_static(inputs):
    """Host-side prep of weight-derived tensors (shared across cores)."""
    dw_w = np.asarray(inputs["dw_weight"], np.float32).reshape(EC, KS, KS)
    dw_b = np.asarray(inputs["dw_bias"], np.float32)
    band = np.zeros((KS, P, EC, P), np.float32)
    for i in range(KS):
        for dq in range(-PAD, PAD + 1):
            j = dq + PAD
            qo = np.arange(max(0, -dq), min(P, P - dq))
            band[i, qo + dq, :, qo] = dw_w[:, i, j][None, :]
    band = band.reshape(KS * P, EC, P)
    bias_row = np.tile(dw_b[:, None], (1, P)).reshape(1, EC, P)
    band = np.concatenate([band, bias_row], axis=0)  # [113, EC, P]
    # reorder flat channels to (quarter, expert, channel-in-quarter)
    CQ = C // 4
    order = [e * C + qd * CQ + t for qd in range(4) for e in range(E)
             for t in range(CQ)]
    band = band[:, order, :].reshape(KS * P + 1, EC * P)
    return {
        "dw_band": np.ascontiguousarray(band),
        "gn_w": np.asarray(inputs["gn_weight"], np.float32),
        "gn_b": np.asarray(inputs["gn_bias"], np.float32),
        "pw_in_w": np.asarray(inputs["pw_in_weight"], np.float32),
        "pw_in_b": np.asarray(inputs["pw_in_bias"], np.float32),
        "pw_out_w": np.asarray(inputs["pw_out_weight"], np.float32),
        "pw_out_b": np.asarray(inputs["pw_out_bias"], np.float32),
        "cond_w": np.asarray(inputs["cond_w"], np.float32),
        "cond_b": np.asarray(inputs["cond_b"], np.float32),
    }


def _prep_core(x_k, cond_k):
    """Per-core prep: shifted-transposed rhs for dw, (q,l,p)-ordered x/cond."""
    xt = x_k.transpose(3, 0, 1, 2)  # [q, c, l, p]
    rhs = np.zeros((KS, P, C, L, P), np.float32)
    for i in range(KS):
        a, b = max(0, PAD - i), min(P, P + PAD - i)
        rhs[i, :, :, :, a:b] = xt[:, :, :, a + i - PAD:b + i - PAD]
    rhs = rhs.reshape(KS * P, C * LP)
    rhs = np.concatenate([rhs, np.ones((1, C * LP), np.float32)], axis=0)
    return {
        "dw_rhs": np.ascontiguousarray(rhs),
        "x_qlp": np.ascontiguousarray(
            x_k.transpose(0, 3, 1, 2).reshape(C, S)),
        "cond_qlp": np.ascontiguousarray(
            cond_k.transpose(0, 3, 1, 2).reshape(CONDC, S)),
    }


def kernel(**inputs):
    from concourse.bass_utils import run_bass_kernel_spmd

    nc = _get_built()
    x = np.asarray(inputs["x"], dtype=np.float32)
    cond = np.asarray(inputs["cond"], dtype=np.float32)
    base = _prep_static(inputs)
    in_maps = []
    for k in range(N):
        m = dict(base)
        m.update(_prep_core(x[k], cond[k]))
        in_maps.append(m)
    res = run_bass_kernel_spmd(nc, in_maps, list(range(N)))
    out = np.empty((N, E, C, L, P, P), dtype=np.float32)
    for k in range(N):
        out[k] = res.results[k]["out"].reshape(E, C, L, P, P)
    return out
